# revision 29
# baseline (speedup 1.0000x reference)
"""GPT-style transformer forward on 8 Trainium2 NeuronCores.

Sharding: data-parallel over batch (2 groups of 4 cores), tensor-parallel
within each group (heads / FFN hidden / vocab columns split 4 ways).
Device activations are feature-major [feature, token] so all matmuls run
without transposes.

Schedule: per layer the token dim is split into two 512-blocks; each
block's AllReduce is issued while the other block computes (attention,
FFN, or the next layer's QKV), hiding collective latency.  LN gamma/beta
and all linear biases are folded into weights / eviction biases host-side,
so the device residual+LN is: fused residual-scale-add (gpsimd STT) ->
sum/sumsq ones-matmuls -> rsqrt -> two broadcast-apply ops.  Causal
attention computes only unmasked column ranges; diagonal tiles are masked
in-place with affine_select.  The log-softmax denominator is reduced on
device (per-core sum-exp) and the final subtract is folded into the
host-side unshard.
"""

import os
from contextlib import ExitStack

import numpy as np
import ml_dtypes

import concourse.bass as bass
import concourse.mybir as mybir
import concourse.tile as tile
from concourse.bass_utils import run_bass_kernel_spmd
from concourse.vector_clock import ScopedClock


def _drain_and_barrier(self, tick_clock, wait_clock):
    """The walrus build here encodes Drain/NoOp as TPB_CTRL with at most one
    sync-wait slot; Tile's stock tail attaches all outstanding waits to the
    Drain and fails codegen. Split the waits one-per-NOP instead."""
    nop_inst = self.nc.sync.nop(nofuse=True)
    wait_clock.add_sem_waits(nop_inst.ins, ScopedClock({None: tick_clock.global_clock}))
    si = nop_inst.ins.sync_info
    if si is not None and len(si.on_wait) > 1:
        waits = list(si.on_wait)
        nop_inst.ins.sync_info = mybir.SyncInfo(on_wait=waits[:1], on_update=list(si.on_update))
        for w in waits[1:]:
            n2 = self.nc.sync.nop(nofuse=True)
            n2.ins.sync_info = mybir.SyncInfo(on_wait=[w], on_update=[])
    self.nc.sync.drain()
    self.nc.all_engine_barrier()
    assert self.sems is not None
    popped = self.nc._tile_sem_poison_stack.pop()
    assert popped is self._sem_poison
    self.nc.clear_and_free_semaphores(list(self.sems.allocated().values()))
    self.nc.all_engine_barrier()


tile.TileContext._drain_and_barrier = _drain_and_barrier

_MAX_WAITS = 1  # this walrus build caps sync-waits per instruction


def split_sync_waits(nc):
    """Hoist excess on_wait entries onto same-engine NOPs inserted before the
    instruction (engine queues execute in program order, so semantics hold)."""
    n = 0
    for bb in nc.main_func.blocks:
        insts = bb.instructions
        new_list = []
        for inst in insts:
            si = getattr(inst, "sync_info", None)
            if si is not None and len(si.on_wait) > _MAX_WAITS:
                waits = list(si.on_wait)
                for w in waits[:-_MAX_WAITS]:
                    n += 1
                    new_list.append(mybir.InstNoOp(
                        name=f"{inst.name}-sw{n}",
                        sync_info=mybir.SyncInfo(on_wait=[w], on_update=[]),
                        bass_nofuse=True,
                        engine=inst.engine,
                    ))
                inst.sync_info = mybir.SyncInfo(
                    on_wait=waits[-_MAX_WAITS:], on_update=list(si.on_update)
                )
            new_list.append(inst)
        if len(new_list) != len(insts):
            bb.instructions[:] = new_list
    return n


# Model dims (hardcoded per problem spec)
L_FULL, H, D, V, SMAX = 8, 16, 1024, 32000, 1024
DH = D // H          # 64
FF = 4 * D           # 4096
B, S = 2, 1024
T = S                # tokens per group (one batch element per group)
TP = 4               # tensor-parallel degree within a group
HL = H // TP         # 4 local heads
FFL = FF // TP       # 1024 local FFN cols
VL = V // TP         # 8000 local vocab cols
VLP = 8064           # padded to 63*128
NVM = VLP // 128     # 63 vocab m-tiles
EPS = 1e-5
KT = D // 128        # 8 k-tiles over model dim
NB = T // 512        # 2 token blocks of 512

BF = mybir.dt.bfloat16
F32 = mybir.dt.float32
AF = mybir.ActivationFunctionType
ALU = mybir.AluOpType

RG = [[0, 1, 2, 3], [4, 5, 6, 7]]

N_LAYERS = int(os.environ.get("BASS_GPT_LAYERS", str(L_FULL)))
SKIP_FINAL = os.environ.get("BASS_GPT_SKIP_FINAL", "0") == "1"


def _r2(ap):
    """[ (kt p) n ] -> [p kt n] view of a DRAM 2-D tensor (p=128)."""
    return ap.rearrange("(kt p) n -> p kt n", p=128)


def build_program():
    nc = bass.Bass("TRN2")

    # ---- DRAM parameters (per-core shards; gamma/beta/bias folded host-side) ----
    h0T = nc.declare_dram_parameter("h0T", [D, T], BF, isOutput=False)
    wqkv = nc.declare_dram_parameter("wqkv", [N_LAYERS, D, 3 * HL * DH], BF, isOutput=False)
    bqk = nc.declare_dram_parameter("bqk", [N_LAYERS, 4 * 128], F32, isOutput=False)
    wo = nc.declare_dram_parameter("wo", [N_LAYERS, HL * DH, D], BF, isOutput=False)
    evb1 = nc.declare_dram_parameter("evb1", [N_LAYERS, D], F32, isOutput=False)
    evb2 = nc.declare_dram_parameter("evb2", [N_LAYERS, D], F32, isOutput=False)
    resga = nc.declare_dram_parameter("resga", [N_LAYERS, D], F32, isOutput=False)
    resgb = nc.declare_dram_parameter("resgb", [N_LAYERS, D], F32, isOutput=False)
    w1 = nc.declare_dram_parameter("w1", [N_LAYERS, D, FFL], BF, isOutput=False)
    b1p = nc.declare_dram_parameter("b1p", [N_LAYERS, FFL], F32, isOutput=False)
    w2 = nc.declare_dram_parameter("w2", [N_LAYERS, FFL, D], BF, isOutput=False)
    wout = nc.declare_dram_parameter("wout", [D, VLP], BF, isOutput=False)
    bout = nc.declare_dram_parameter("bout", [VLP], F32, isOutput=False)
    out = nc.declare_dram_parameter("out", [VLP, T], F32, isOutput=True)
    sumexp = nc.declare_dram_parameter("sumexp", [NB, 512], F32, isOutput=True)

    with ExitStack() as ctx:
        tc = ctx.enter_context(tile.TileContext(nc))

        const = ctx.enter_context(tc.tile_pool(name="const", bufs=1))
        hpool = ctx.enter_context(tc.tile_pool(name="hpool", bufs=1))
        apool = ctx.enter_context(tc.tile_pool(name="apool", bufs=1))
        epool = ctx.enter_context(tc.tile_pool(name="epool", bufs=2))
        s1pool = ctx.enter_context(tc.tile_pool(name="s1pool", bufs=1))
        wq_pool = ctx.enter_context(tc.tile_pool(name="wq_pool", bufs=2))
        wf_pool = ctx.enter_context(tc.tile_pool(name="wf_pool", bufs=1))
        wch_pool = ctx.enter_context(tc.tile_pool(name="wch_pool", bufs=3))
        bpool = ctx.enter_context(tc.tile_pool(name="bpool", bufs=2))
        spool = ctx.enter_context(tc.tile_pool(name="spool", bufs=2))
        rpool = ctx.enter_context(tc.tile_pool(name="rpool", bufs=2))
        fpool = ctx.enter_context(tc.tile_pool(name="fpool", bufs=1))

        mm_psum = ctx.enter_context(tc.tile_pool(name="mm_psum", bufs=3, space="PSUM"))
        o_psum = ctx.enter_context(tc.tile_pool(name="o_psum", bufs=2, space="PSUM"))
        st_psum = ctx.enter_context(tc.tile_pool(name="st_psum", bufs=2, space="PSUM"))
        bc_psum = ctx.enter_context(tc.tile_pool(name="bc_psum", bufs=1, space="PSUM"))

        dram = ctx.enter_context(tc.tile_pool(name="dram", bufs=3, space="DRAM"))

        # ---- constants ----
        ones_d = const.tile([128, 1], BF)       # partition-sum lhsT, scaled 1/D (LN stats)
        nc.vector.memset(ones_d, 1.0 / D)
        ones_1 = const.tile([128, 1], BF)       # partition-sum lhsT (softmax denominator)
        nc.vector.memset(ones_1, 1.0)
        ones_m = const.tile([1, 128], BF)       # broadcast lhsT (K=1, M=128)
        nc.vector.memset(ones_m, 1.0)
        negones_m = const.tile([1, 128], BF)    # negated broadcast lhsT
        nc.vector.memset(negones_m, -1.0)
        eps_sb = const.tile([1, 1], F32)
        nc.vector.memset(eps_sb, float(EPS))

        # ---- persistent activation state ----
        hb = hpool.tile([128, KT, T], BF)       # residual stream (feature-major, pre-gamma/beta)
        qk_sb = apool.tile([128, 2, 2, T], BF)  # [part, q/k, head-pair, t]
        vaug = apool.tile([128, KT, HL, 65], BF)  # token-major V + ones col
        oT = apool.tile([128, 2, T], BF)        # attn head outputs (feature-major, normalized)
        f1 = fpool.tile([128, KT, T], BF)       # FFN hidden (local)

        nc.sync.dma_start(hb, _r2(h0T))         # h0 straight into the residual stream
        nc.vector.memset(vaug[:, :, :, 64:65], 1.0)

        def ln_block(nb, ar_out, gcol):
            """hb[:, :, tsl] <- normalize(hb * gcol + AR result) (token block nb)."""
            tsl = slice(nb * 512, (nb + 1) * 512)
            arb = s1pool.tile([128, KT, 512], BF, tag="arb")
            nc.sync.dma_start(arb, _r2(ar_out))
            xb = s1pool.tile([128, KT, 512], BF, tag="xb")
            ps_st = st_psum.tile([65, 512], F32, tag="st")
            for kt in range(KT):
                nc.vector.scalar_tensor_tensor(
                    out=xb[:, kt, :], in0=hb[:, kt, tsl],
                    scalar=gcol[:, kt : kt + 1], in1=arb[:, kt, :],
                    op0=ALU.mult, op1=ALU.add,
                )
                xsq = spool.tile([128, 512], BF, tag="xsq")
                nc.scalar.activation(xsq, xb[:, kt, :], AF.Square)
                nc.tensor.matmul(ps_st[0:1, :], ones_d, xb[:, kt, :],
                                 start=(kt == 0), stop=(kt == KT - 1),
                                 skip_group_check=True)
                nc.tensor.matmul(ps_st[64:65, :], ones_d, xsq,
                                 start=(kt == 0), stop=(kt == KT - 1),
                                 skip_group_check=True)
            # u = m2 - mu^2 ; a = rsqrt(u + eps) = exp(-0.5 ln(u + eps))
            negs1r = rpool.tile([1, 512], BF, tag="negs1r")
            nc.vector.tensor_scalar(out=negs1r, in0=ps_st[0:1, :], scalar1=-1.0,
                                    scalar2=None, op0=ALU.mult)
            t0n = rpool.tile([1, 512], F32, tag="t0n")
            nc.vector.tensor_mul(t0n, negs1r, ps_st[0:1, :])       # -mu^2
            u = rpool.tile([1, 512], F32, tag="u")
            nc.vector.tensor_add(u, t0n, ps_st[64:65, :])
            lnr = rpool.tile([1, 512], F32, tag="lnr")
            nc.scalar.activation(lnr, u, AF.Ln, bias=eps_sb[0:1, 0:1])
            a_row = rpool.tile([1, 512], BF, tag="arow")
            nc.scalar.activation(a_row, lnr, AF.Exp, scale=-0.5)
            # broadcast -mu and a across partitions via K=1 matmuls, stage in SBUF
            ps_nm = bc_psum.tile([128, 512], F32, tag="bc")
            nc.tensor.matmul(ps_nm, ones_m, negs1r, start=True, stop=True)
            nmb = spool.tile([128, 512], BF, tag="nmb")
            nc.scalar.activation(nmb, ps_nm, AF.Identity)
            ps_ab = bc_psum.tile([128, 512], F32, tag="bc")
            nc.tensor.matmul(ps_ab, ones_m, a_row, start=True, stop=True)
            ab = spool.tile([128, 512], BF, tag="ab")
            nc.scalar.activation(ab, ps_ab, AF.Identity)
            for kt in range(KT):
                tt = spool.tile([128, 512], BF, tag="lnt")
                # gpsimd TT runs ~2.7x slower than vector bf16; give it 1/4
                e_add = nc.gpsimd if kt % 4 == 3 else nc.vector
                e_mul = nc.gpsimd if kt % 4 == 1 else nc.vector
                e_add.tensor_add(tt, xb[:, kt, :], nmb)
                e_mul.tensor_mul(hb[:, kt, tsl], tt, ab)

        def evict(ps, out_ap, col=None, relu=False, eng="scalar"):
            """PSUM -> SBUF eviction with optional per-partition bias / relu.
            (gpsimd cannot read PSUM, so only scalar/vector qualify.)"""
            if eng == "scalar":
                nc.scalar.activation(out_ap, ps, AF.Relu if relu else AF.Identity,
                                     bias=col if col is not None else 0.0)
            else:
                e = nc.vector
                if relu:
                    e.tensor_scalar(out=out_ap, in0=ps,
                                    scalar1=col if col is not None else 0.0,
                                    scalar2=0.0, op0=ALU.add, op1=ALU.max)
                elif col is not None:
                    e.tensor_scalar(out=out_ap, in0=ps, scalar1=col, scalar2=None,
                                    op0=ALU.add)
                else:
                    e.tensor_copy(out_ap, ps)

        RR = ("scalar", "vector")

        def qkv_block(nb, wqkv_sb, bqk_sb):
            tsl = slice(nb * 512, (nb + 1) * 512)
            for io in range(2):        # 0=q, 1=k  (feature-major out)
                for mt in range(2):    # head pair
                    mcol = (io * 2 + mt) * 128
                    ps = mm_psum.tile([128, 512], F32, tag="mm")
                    for kt in range(KT):
                        nc.tensor.matmul(
                            ps, wqkv_sb[:, kt, mcol : mcol + 128], hb[:, kt, tsl],
                            start=(kt == 0), stop=(kt == KT - 1),
                        )
                    evict(ps, qk_sb[:, io, mt, tsl],
                          col=bqk_sb[:, io * 2 + mt : io * 2 + mt + 1],
                          eng=RR[(io * 2 + mt) % 2])
            for tm in range(nb * 4, nb * 4 + 4):   # v, token-major
                ps = mm_psum.tile([128, 512], F32, tag="mm")
                for kt in range(KT):
                    nc.tensor.matmul(
                        ps[:, 0:256], hb[:, kt, tm * 128 : (tm + 1) * 128],
                        wqkv_sb[:, kt, 512:768],
                        start=(kt == 0), stop=(kt == KT - 1),
                    )
                evict(ps[:, 0:256].rearrange("p (h e) -> p h e", h=HL),
                      vaug[:, tm, :, 0:64], eng=RR[tm % 2])

        def attn_block(blk):
            t1base = blk * 512
            t2max = 4 * (blk + 1)
            for h in range(HL):
                prow = slice(64 * (h % 2), 64 * (h % 2) + 64)
                hm = h // 2
                et = epool.tile([128, 8, 512], BF, tag="eT")
                for t2t in range(t2max):
                    a = max(0, 128 * (t2t - 4 * blk))
                    ps = mm_psum.tile([128, 512], F32, tag="mm")
                    nc.tensor.matmul(
                        ps[:, a:],
                        qk_sb[prow, 1, hm, t2t * 128 : (t2t + 1) * 128],
                        qk_sb[prow, 0, hm, t1base + a : t1base + 512],
                        start=True, stop=True,
                    )
                    nc.scalar.activation(et[:, t2t, a:], ps[:, a:], AF.Exp, scale=0.125)
                    j = t2t - 4 * blk
                    if 0 <= j <= 3:
                        # diagonal 128x128 triangle: keep where t1 - t2 >= 0
                        nc.gpsimd.affine_select(
                            out=et[:, t2t, a : a + 128], in_=et[:, t2t, a : a + 128],
                            compare_op=ALU.is_ge, fill=0.0,
                            base=0, pattern=[[1, 128]], channel_multiplier=-1,
                        )
                ps_o = o_psum.tile([65, 512], F32, tag="o")
                for t2t in range(t2max):
                    a = max(0, 128 * (t2t - 4 * blk))
                    nc.tensor.matmul(
                        ps_o[:, a:], vaug[:, t2t, h, :], et[:, t2t, a:],
                        start=(t2t == 0), stop=(t2t == t2max - 1),
                        skip_group_check=True,
                    )
                # 1/den = exp(-ln(den)); broadcast over the 64 head dims via PE
                lnd = rpool.tile([1, 512], F32, tag="lnd")
                nc.scalar.activation(lnd, ps_o[64:65, :], AF.Ln)
                rec = rpool.tile([1, 512], BF, tag="rec")
                nc.scalar.activation(rec, lnd, AF.Exp, scale=-1.0)
                ps_rb = bc_psum.tile([128, 512], F32, tag="bc")
                nc.tensor.matmul(ps_rb[0:64, :], ones_m[:, 0:64], rec,
                                 start=True, stop=True)
                osb = spool.tile([64, 512], BF, tag="osb")
                nc.vector.tensor_copy(osb, ps_o[0:64, :])
                nc.vector.tensor_mul(oT[prow, hm, t1base : t1base + 512],
                                     osb, ps_rb[0:64, :])

        def outproj_block(nb, wo_sb, ev1_sb):
            tsl = slice(nb * 512, (nb + 1) * 512)
            ar_in = dram.tile([D, 512], BF, tag="arin")
            for mt in range(KT):
                ps = mm_psum.tile([128, 512], F32, tag="mm")
                for kt in range(2):
                    nc.tensor.matmul(
                        ps, wo_sb[:, kt, mt * 128 : (mt + 1) * 128], oT[:, kt, tsl],
                        start=(kt == 0), stop=(kt == 1),
                    )
                ob = spool.tile([128, 512], BF, tag="ob")
                evict(ps, ob, col=ev1_sb[:, mt : mt + 1], eng=RR[mt % 2])
                nc.sync.dma_start(ar_in[mt * 128 : (mt + 1) * 128, :], ob)
            ar_out = dram.tile([D, 512], BF, tag="arout")
            nc.gpsimd.collective_compute(
                "AllReduce", ALU.add, replica_groups=RG,
                ins=[ar_in.opt()], outs=[ar_out.opt()],
            )
            return ar_out

        def ffn_block(nb, w1_sb, w2_sb, b1_sb, ev2_sb):
            tsl = slice(nb * 512, (nb + 1) * 512)
            for mt in range(KT):
                ps = mm_psum.tile([128, 512], F32, tag="mm")
                for kt in range(KT):
                    nc.tensor.matmul(
                        ps, w1_sb[:, kt, mt * 128 : (mt + 1) * 128], hb[:, kt, tsl],
                        start=(kt == 0), stop=(kt == KT - 1),
                    )
                evict(ps, f1[:, mt, tsl], col=b1_sb[:, mt : mt + 1], relu=True,
                      eng=RR[mt % 2])
            ar_in = dram.tile([D, 512], BF, tag="arin")
            for mt in range(KT):
                ps = mm_psum.tile([128, 512], F32, tag="mm")
                for kt in range(KT):
                    nc.tensor.matmul(
                        ps, w2_sb[:, kt, mt * 128 : (mt + 1) * 128], f1[:, kt, tsl],
                        start=(kt == 0), stop=(kt == KT - 1),
                    )
                ob = spool.tile([128, 512], BF, tag="ob")
                evict(ps, ob, col=ev2_sb[:, mt : mt + 1], eng=RR[(mt + 1) % 2])
                nc.sync.dma_start(ar_in[mt * 128 : (mt + 1) * 128, :], ob)
            ar_out = dram.tile([D, 512], BF, tag="arout")
            nc.gpsimd.collective_compute(
                "AllReduce", ALU.add, replica_groups=RG,
                ins=[ar_in.opt()], outs=[ar_out.opt()],
            )
            return ar_out

        def load_cols(param, l, tag):
            t = bpool.tile([128, KT], F32, tag=tag)
            nc.sync.dma_start(t, param[l].rearrange("(m p) -> p m", p=128))
            return t

        # ---- layer loop; FFN AllReduces carried into the next iteration ----
        ar_ffn = [None, None]
        gb_prev = None
        for l in range(N_LAYERS):
            wqkv_sb = wq_pool.tile([128, KT, 768], BF, tag="wqkv")
            nc.sync.dma_start(wqkv_sb, _r2(wqkv[l]))
            wo_sb = wq_pool.tile([128, 2, D], BF, tag="wo")
            nc.sync.dma_start(wo_sb, _r2(wo[l]))
            w1_sb = wf_pool.tile([128, KT, FFL], BF, tag="w1")
            nc.sync.dma_start(w1_sb, _r2(w1[l]))
            w2_sb = wf_pool.tile([128, KT, D], BF, tag="w2")
            nc.sync.dma_start(w2_sb, _r2(w2[l]))
            bqk_sb = bpool.tile([128, 4], F32, tag="bqk")
            nc.sync.dma_start(bqk_sb, bqk[l].rearrange("(m p) -> p m", p=128))
            ev1_sb = load_cols(evb1, l, "ev1")
            ev2_sb = load_cols(evb2, l, "ev2")
            ga_sb = load_cols(resga, l, "ga")     # gamma2[l-1] (ones at l=0)
            gb_sb = load_cols(resgb, l, "gb")     # gamma1[l]
            b1_sb = load_cols(b1p, l, "b1")

            # LN2 of previous layer (consumes prev FFN ARs), then this layer's
            # QKV + attention + out-proj, per token block, so each AllReduce is
            # covered by the other block's compute
            ar_attn = [None, None]
            for nb in range(NB):
                if l > 0:
                    with nc.named_scope(f"L{l}.ln2p.{nb}", notify=True):
                        ln_block(nb, ar_ffn[nb], gb_prev)
                with nc.named_scope(f"L{l}.qkv.{nb}", notify=True):
                    qkv_block(nb, wqkv_sb, bqk_sb)
                with nc.named_scope(f"L{l}.attn.{nb}", notify=True):
                    attn_block(nb)
                with nc.named_scope(f"L{l}.oproj.{nb}", notify=True):
                    ar_attn[nb] = outproj_block(nb, wo_sb, ev1_sb)

            for nb in range(NB):
                with nc.named_scope(f"L{l}.ln1.{nb}", notify=True):
                    ln_block(nb, ar_attn[nb], ga_sb)
                with nc.named_scope(f"L{l}.ffn.{nb}", notify=True):
                    ar_ffn[nb] = ffn_block(nb, w1_sb, w2_sb, b1_sb, ev2_sb)
            gb_prev = gb_sb

        # ---- final LN2, vocab projection, per-core sum-exp ----
        bout_sb = const.tile([128, NVM], F32)
        nc.sync.dma_start(bout_sb, bout.rearrange("(m p) -> p m", p=128))
        for nb in range(NB):
            ln_block(nb, ar_ffn[nb], gb_prev)
            tsl = slice(nb * 512, (nb + 1) * 512)
            ps_acc_t = st_psum.tile([65, 512], F32, tag="st")
            ps_acc = ps_acc_t[0:1, :]
            for vm in range(NVM):
                wv_sb = wch_pool.tile([128, KT, 128], BF, tag="wch")
                nc.sync.dma_start(wv_sb, _r2(wout)[:, :, vm * 128 : (vm + 1) * 128])
                ps = mm_psum.tile([128, 512], F32, tag="mm")
                for kt in range(KT):
                    nc.tensor.matmul(
                        ps, wv_sb[:, kt, :], hb[:, kt, tsl],
                        start=(kt == 0), stop=(kt == KT - 1),
                    )
                outf = spool.tile([128, 512], F32, tag="outf")
                evict(ps, outf, col=bout_sb[:, vm : vm + 1], eng=RR[vm % 2])
                nc.sync.dma_start(out[vm * 128 : (vm + 1) * 128, tsl], outf)
                eb = spool.tile([128, 512], BF, tag="eb")
                nc.scalar.activation(eb, ps, AF.Exp, bias=bout_sb[:, vm : vm + 1])
                nc.tensor.matmul(
                    ps_acc, ones_1, eb,
                    start=(vm == 0), stop=(vm == NVM - 1), skip_group_check=True,
                )
            se_row = rpool.tile([1, 512], F32, tag="serow")
            nc.vector.tensor_copy(se_row, ps_acc)
            nc.sync.dma_start(sumexp[nb : nb + 1, :], se_row)

    nsplit = split_sync_waits(nc)
    print(f"split_sync_waits: {nsplit} NOPs inserted")
    return nc


def _bf16(a):
    return np.asarray(a, dtype=ml_dtypes.bfloat16)


def make_in_maps(x, tok_emb, pos_emb, wq, bq, wk, bk, wv, bv, wo, bo,
                 ln1_g, ln1_b, w1, b1, w2, b2, ln2_g, ln2_b, w_out, b_out):
    """Shard full inputs -> per-core input maps (with host-side folds)."""
    LE = wq.shape[0]
    f32 = np.float32
    # gamma2/beta2 of the *previous* layer (identity for layer 0)
    ga = np.concatenate([np.ones((1, D), f32), ln2_g[:-1]], axis=0)   # [L, D]
    be = np.concatenate([np.zeros((1, D), f32), ln2_b[:-1]], axis=0)  # [L, D]
    per_r = []
    for r in range(TP):
        hs = slice(HL * r, HL * (r + 1))
        # per-head weights, head-major concat, gamma-prev scaled rows
        wq_r = wq[:, hs].transpose(0, 2, 1, 3).reshape(LE, D, HL * DH)
        wk_r = wk[:, hs].transpose(0, 2, 1, 3).reshape(LE, D, HL * DH)
        wv_r = wv[:, hs].transpose(0, 2, 1, 3).reshape(LE, D, HL * DH)
        wqkv_r = np.concatenate([wq_r, wk_r, wv_r], axis=2) * ga[:, :, None]
        # bias folds: b' = b + W^T beta_prev
        bq_r = bq[:, hs].reshape(LE, -1) + np.einsum('ldm,ld->lm', wq_r, be)
        bk_r = bk[:, hs].reshape(LE, -1) + np.einsum('ldm,ld->lm', wk_r, be)
        bv_r = bv[:, hs].reshape(LE, -1) + np.einsum('ldm,ld->lm', wv_r, be)
        bqk_r = np.concatenate([bq_r, bk_r], axis=1).astype(f32)      # [L, 512]
        wo_r = wo[:, DH * HL * r : DH * HL * (r + 1), :]              # [L, 256, D]
        # eviction biases (pre-AllReduce, so /TP; plus folded V-bias through wo)
        ev1_r = (bo + be) / TP + np.einsum('lcd,lc->ld', wo_r, bv_r)
        ev2_r = (b2 + ln1_b) / TP
        fs = slice(FFL * r, FFL * (r + 1))
        w1_r = w1[:, :, fs] * ln1_g[:, :, None]
        b1_r = b1[:, fs] + np.einsum('ldm,ld->lm', w1[:, :, fs], ln1_b)
        vs = slice(VL * r, VL * (r + 1))
        wout_r = np.zeros((D, VLP), f32)
        wout_r[:, :VL] = w_out[:, vs] * ln2_g[-1][:, None]
        bout_r = np.full((VLP,), -1e30, f32)
        bout_r[:VL] = b_out[vs] + w_out[:, vs].T @ ln2_b[-1]
        per_r.append(dict(
            wqkv=_bf16(wqkv_r),
            bqk=np.ascontiguousarray(bqk_r),
            wo=_bf16(wo_r),
            evb1=np.ascontiguousarray(ev1_r, f32),
            evb2=np.ascontiguousarray(ev2_r, f32),
            resga=np.ascontiguousarray(ga, f32),
            resgb=np.ascontiguousarray(ln1_g, f32),
            w1=_bf16(w1_r),
            b1p=np.ascontiguousarray(b1_r, f32),
            w2=_bf16(w2[:, fs, :]),
            wout=_bf16(wout_r),
            bout=bout_r,
        ))
    in_maps = []
    for c in range(8):
        g, r = c // TP, c % TP
        emb = tok_emb[x[g]] + pos_emb[:S]          # [S, D]
        m = dict(per_r[r])
        m["h0T"] = _bf16(np.ascontiguousarray(emb.T))
        in_maps.append(m)
    return in_maps


_CACHED = {}


def _install_ntff_shim():
    """Provide antenv.axon_hooks.get_axon_ntff_profile_hook via ctypes on
    libaxon_pjrt.so (this container's trn_rl_repo snapshot lacks the module)."""
    import sys
    import types
    import ctypes
    import contextlib

    if "antenv.axon_hooks" in sys.modules:
        return
    try:
        lib = ctypes.CDLL("/opt/axon/libaxon_pjrt.so")
    except OSError:
        return
    lib.axon_start_nrt_profile.restype = ctypes.c_int64
    lib.axon_start_nrt_profile.argtypes = [ctypes.c_char_p, ctypes.c_size_t]
    lib.axon_stop_nrt_profile.restype = ctypes.c_int64
    lib.axon_stop_nrt_profile.argtypes = [ctypes.c_char_p]

    def get_axon_ntff_profile_hook():
        @contextlib.contextmanager
        def hook(neff_dir, trace_model_indices):
            d = str(neff_dir).encode()
            if lib.axon_start_nrt_profile(d, len(d)) != 0:
                yield
                return
            try:
                yield
            finally:
                lib.axon_stop_nrt_profile(d)

        return hook

    mod = types.ModuleType("antenv.axon_hooks")
    mod.get_axon_ntff_profile_hook = get_axon_ntff_profile_hook
    sys.modules["antenv.axon_hooks"] = mod


def kernel(**inputs):
    inputs = {k: np.asarray(v) for k, v in inputs.items()}
    if "nc" not in _CACHED:
        _CACHED["nc"] = build_program()
    nc = _CACHED["nc"]
    in_maps = make_in_maps(**inputs)
    trace = os.environ.get("BASS_GPT_TRACE", "0") == "1"
    if trace:
        _install_ntff_shim()
    res = run_bass_kernel_spmd(
        nc, in_maps, core_ids=list(range(8)), trace=trace,
    )
    if trace:
        print(f"HW exec time: {res.exec_time_ns} ns")
        _CACHED["last_result"] = res
    results = res.results
    full = np.empty((B, S, V), np.float32)
    for g in range(B):
        se = np.zeros((T,), np.float64)
        for r in range(TP):
            se += results[g * TP + r]["sumexp"].reshape(T).astype(np.float64)
        lse = np.log(se).astype(np.float32)                  # [T]
        for r in range(TP):
            c = g * TP + r
            full[g, :, VL * r : VL * (r + 1)] = (
                results[c]["out"][:VL, :] - lse[None, :]
            ).T
    return full
